# revision 4
# baseline (speedup 1.0000x reference)
"""Trainium2 Bass kernel for GatedActivation (gate-conv3d + sigmoid gating).

Reference computation (see problem):
  x: [2, 120, 48, 48, 48] f32   (channels = 32 scalar + 16*3 (l=1) + 8*5 (l=2))
  w_gate: [24, 120, 5, 5, 5] f32
  g = sigmoid(conv3d(x, w_gate, same padding))         # [2, 24, 48, 48, 48]
  out[:, 0:32]   = relu(x[:, 0:32])
  out[:, 32+3m+d]  = x[:, 32+3m+d]  * g[:, m]          (m in 0..15)
  out[:, 80+5m+d]  = x[:, 80+5m+d]  * g[:, 16+m]       (m in 0..7)

Sharding: 8 cores = batch(2) x X-split(4 slabs of 12 planes).  Each core gets a
host-prepared zero-padded fp16 input slab [128, 16, 52, 52] (= [C, Xin, Ypad,
Zpad]) and produces fp32 [120, 12, 48, 48].

Channels are permuted on the host so the gated block sits at partitions 0:88
(base-0 vector multiply) and the scalar/relu block at 96:128 (legal
32-partition base-96 activation):
    slab rows: [l=1 (48) | l=2 (40) | zero pad (8) | scalar (32)]

On-chip algorithm per core (matmul operands in fp16; fp32 PSUM accumulate):
  - conv is computed as 25 accumulating matmuls per output tile, one per
    (kx, ky) tap, with the kz (z) taps folded into the stationary columns:
      stationary W'[(kx,ky)][cin, kz*24+o], moving x[cin, y-tile, zpad-window]
    -> PSUM[(kz*24+o), y, zpad].  Stationary is padded to 128 columns so the
    fast-weight-load path engages.
  - fixup per tile (all off the tensor engine): PSUM->SBUF fp16 cast copy
    (ACT), 5 z-shift SBUF->SBUF DMAs into 24-partition tiles (SWDGE; DMA APs
    are exempt from the engines' 32-partition alignment rule), a 4-deep DVE
    add chain for the z-tap sum, sigmoid on the 24 gate partitions (ACT),
    and a gate->channel broadcast as two free-dim stride-0 replicating DMAs
    on the sync engine's HWDGE ring.
  - gating multiply on DVE over partitions 0:88, relu of the scalar block at
    partitions 96:128 on ACT, 3 channel-block output DMAs per x-plane that
    un-permute on the way out.

vs the previous version this removes the per-tile esum selector matmul (and
its gpre PSUM banks) from the tensor engine, which measures ~15 us faster;
the conv matmul stream itself is the hard floor (the PE sustains ~2.0-2.2
GHz under full 8-core load, so the 72x25xN=416 fp16 stream is ~375-390 us).
"""

import sys

if "/opt/trn_rl_repo" not in sys.path:
    sys.path.insert(0, "/opt/trn_rl_repo")

import numpy as np

B = 2
C = 120
S = 48          # spatial size
K = 5           # conv kernel size (odd)
PAD = 2
NXS = 4         # x-axis shards
XS = S // NXS   # 12 output x-planes per core
XIN = XS + 2 * PAD   # 16 input planes per core
SP = S + 2 * PAD     # 52, padded y/z extent
YT = 8          # y-tile rows per matmul
NYT = S // YT   # 6 y-tiles per plane
NCO = 24        # gate output channels
NSC = 32        # scalar (relu) channels
CP = 128        # stationary columns incl. pad (FWL wants 128)
CIN = 128       # contraction rows incl. pad (8 zero input channels)
N_CORES = 8
NG = 88         # gated channels in permuted order (48 l1 + 40 l2)
SC0 = 96        # permuted base of the scalar block

_CACHE = {}


def _build(reps=1):
    import contextlib

    import concourse.tile as tile
    from concourse import bacc, mybir

    f32 = mybir.dt.float32
    f16 = mybir.dt.float16

    nc = bacc.Bacc("TRN2", target_bir_lowering=False, debug=False,
                   num_devices=N_CORES)
    xs_d = nc.dram_tensor("xs", [CIN, XIN, SP, SP], f16, kind="ExternalInput").ap()
    wst_d = nc.dram_tensor("wst", [CIN, K * K, CP], f16, kind="ExternalInput").ap()
    y_d = nc.dram_tensor("y", [C, XS, S, S], f32, kind="ExternalOutput").ap()

    with tile.TileContext(nc) as tc:
        with tc.tile_pool(name="wpool", bufs=1) as wpool, \
             tc.tile_pool(name="planes", bufs=11) as plane_pool, \
             tc.tile_pool(name="convps", bufs=6, space="PSUM") as conv_pool, \
             tc.tile_pool(name="sshift", bufs=4) as sshift_pool, \
             tc.tile_pool(name="stap", bufs=4) as stap_pool, \
             tc.tile_pool(name="gs24p", bufs=4) as gs24_pool, \
             tc.tile_pool(name="gsigp", bufs=4) as gsig_pool, \
             tc.tile_pool(name="outpl", bufs=2) as out_pool:

            wst_t = wpool.tile([CIN, K * K, CP], f16)
            nc.sync.dma_start(wst_t[:], wst_d[:])

            planes = {}
            planes_f = {}

            def load_plane(s):
                t = plane_pool.tile([CIN, SP, SP], f16, tag="plane", name=f"plane{s}")
                nc.sync.dma_start(t[:], xs_d[:, s])
                planes[s] = t
                planes_f[s] = t[:].rearrange("p y z -> p (y z)")

            outplanes = {}

            def emit_fixup(p, k, cps):
                # PSUM -> SBUF cast copy (fp16)
                ss = sshift_pool.tile([CIN, YT, SP], f16)
                nc.scalar.copy(ss[:].rearrange("p y z -> p (y z)"), cps[:, :])
                # 5 parallel z-shift DMAs to 24-partition tiles, then a
                # 4-deep DVE add chain (in-place) for the z-tap sum.
                staps = []
                for c in range(K):
                    t = stap_pool.tile([NCO, YT, S], f16, tag=f"stap{c}",
                                       name=f"stap{c}")
                    nc.gpsimd.dma_start(
                        t[:], ss[NCO * c:NCO * (c + 1), :, c:c + S])
                    staps.append(t)
                stap = staps[0]
                for c in range(1, K):
                    nc.vector.tensor_add(stap[:], stap[:], staps[c][:])
                gs24 = gs24_pool.tile([NCO, YT, S], f16)
                nc.scalar.activation(gs24[:], stap[:],
                                     mybir.ActivationFunctionType.Sigmoid)
                # gate -> channel broadcast (replicate 3x for l=1, 5x for l=2)
                gsig = gsig_pool.tile([NG, YT, S], f16)
                rep1 = gs24[0:16].rearrange("m y z -> m (y z)") \
                    .unsqueeze(1).broadcast_to([16, 3, YT * S])
                nc.sync.dma_start(gsig[0:48].rearrange("p y z -> p (y z)"), rep1)
                rep2 = gs24[16:24].rearrange("m y z -> m (y z)") \
                    .unsqueeze(1).broadcast_to([8, 5, YT * S])
                nc.sync.dma_start(gsig[48:88].rearrange("p y z -> p (y z)"), rep2)

                if k == 0:
                    outplanes[p] = out_pool.tile([CIN, S, S], f32, tag="outplane",
                                                 name=f"outplane{p}")
                op_t = outplanes[p]
                xc = planes[p + PAD]  # center plane (kx = PAD)
                ys = k * YT
                nc.vector.tensor_mul(
                    op_t[0:NG, ys:ys + YT, :],
                    xc[0:NG, ys + PAD:ys + PAD + YT, PAD:PAD + S],
                    gsig[:, :, :])
                nc.scalar.activation(
                    op_t[SC0:CIN, ys:ys + YT, :],
                    xc[SC0:CIN, ys + PAD:ys + PAD + YT, PAD:PAD + S],
                    mybir.ActivationFunctionType.Relu)
                if k == NYT - 1:
                    # un-permute channel blocks on the way out
                    nc.sync.dma_start(y_d[NSC:NSC + 48, p], op_t[0:48])
                    nc.sync.dma_start(y_d[NSC + 48:C, p], op_t[48:NG])
                    nc.sync.dma_start(y_d[0:NSC, p], op_t[SC0:SC0 + NSC])
                    del outplanes[p]

            rep_ctx = tc.For_i(0, reps, 1) if reps > 1 else contextlib.nullcontext()
            with rep_ctx:
                for s in range(K):
                    load_plane(s)

                pending = []
                for p in range(XS):
                    for k in range(NYT):
                        if k == 0 and p + K < XIN:
                            load_plane(p + K)
                        cps = conv_pool.tile([CP, YT * SP], f32)
                        for a in range(K):
                            for b in range(K):
                                st = (k * YT + b) * SP
                                nc.tensor.matmul(
                                    cps[:],
                                    wst_t[:, a * K + b, :],
                                    planes_f[p + a][:, st:st + YT * SP],
                                    start=(a == 0 and b == 0),
                                    stop=(a == K - 1 and b == K - 1))
                        pending.append((p, k, cps))
                        if len(pending) > 3:
                            emit_fixup(*pending.pop(0))
                for args in pending:
                    emit_fixup(*args)

    nc.compile()
    return nc


# permuted channel order: [32:80 (l1), 80:120 (l2)] -> 0:88, scalars -> 96:128
_PERM = np.concatenate([np.arange(32, 80), np.arange(80, 120)])


def _host_inputs(x, w_gate):
    """Build the 8 per-core input maps (matmul operands pre-cast to fp16)."""
    x = np.ascontiguousarray(x, dtype=np.float32)
    w_gate = np.ascontiguousarray(w_gate, dtype=np.float32)

    # stationary: rows permuted like the slab; Wst[i', a*K+b, c*24+o]
    wst = np.transpose(w_gate, (1, 2, 3, 4, 0)).reshape(C, K * K, K * NCO)
    wstp = np.zeros((CIN, K * K, CP), dtype=np.float16)
    wstp[0:NG, :, :K * NCO] = wst[_PERM].astype(np.float16)
    wstp[SC0:SC0 + NSC, :, :K * NCO] = wst[0:NSC].astype(np.float16)

    in_maps = []
    for i in range(N_CORES):
        b = i // NXS
        x0 = (i % NXS) * XS
        slab = np.zeros((CIN, XIN, SP, SP), dtype=np.float16)
        s0 = max(0, x0 - PAD)
        s1 = min(S, x0 + XS + PAD)
        d0 = s0 - (x0 - PAD)
        xx = x[b, :, s0:s1].astype(np.float16)
        slab[0:NG, d0:d0 + (s1 - s0), PAD:PAD + S, PAD:PAD + S] = xx[_PERM]
        slab[SC0:SC0 + NSC, d0:d0 + (s1 - s0), PAD:PAD + S, PAD:PAD + S] = xx[0:NSC]
        in_maps.append({"xs": slab, "wst": wstp})
    return in_maps


def kernel(x, w_gate):
    import time

    from concourse.bass_utils import run_bass_kernel_spmd

    if "nc" not in _CACHE:
        _CACHE["nc"] = _build()
    nc = _CACHE["nc"]

    in_maps = _host_inputs(x, w_gate)
    last_err = None
    for attempt in range(3):
        try:
            res = run_bass_kernel_spmd(nc, in_maps, core_ids=list(range(N_CORES)))
            break
        except Exception as e:  # transient NRT device wedges recover on retry
            last_err = e
            time.sleep(5.0)
    else:
        raise last_err
    kernel._last_results = res

    out = np.empty((B, C, S, S, S), dtype=np.float32)
    for i in range(N_CORES):
        b = i // NXS
        x0 = (i % NXS) * XS
        out[b, :, x0:x0 + XS] = res.results[i]["y"]
    return out


# revision 9
# speedup vs baseline: 1.0831x; 1.0831x over previous
"""Trainium2 Bass kernel for GatedActivation (gate-conv3d + sigmoid gating).

Reference computation (see problem):
  x: [2, 120, 48, 48, 48] f32   (channels = 32 scalar + 16*3 (l=1) + 8*5 (l=2))
  w_gate: [24, 120, 5, 5, 5] f32
  g = sigmoid(conv3d(x, w_gate, same padding))         # [2, 24, 48, 48, 48]
  out[:, 0:32]   = relu(x[:, 0:32])
  out[:, 32+3m+d]  = x[:, 32+3m+d]  * g[:, m]          (m in 0..15)
  out[:, 80+5m+d]  = x[:, 80+5m+d]  * g[:, 16+m]       (m in 0..7)

Sharding: 8 cores = batch(2) x X-split(4 slabs of 12 planes).  Each core gets a
host-prepared zero-padded fp16 input slab [128, 16, 52, 52] (= [C, Xin, Ypad,
Zpad]) and produces fp32 [120, 12, 48, 48].

Channels are permuted on the host so the gated block sits at partitions 0:88
(base-0 vector multiply) and the scalar/relu block at 96:128 (legal
32-partition base-96 activation):
    slab rows: [l=1 (48) | l=2 (40) | zero pad (8) | scalar (32)]

On-chip algorithm per core (matmul operands in fp16; fp32 PSUM accumulate):
  - conv is computed as 25 accumulating matmuls per output tile, one per
    (kx, ky) tap, with the kz (z) taps folded into the stationary columns:
      stationary W'[(kx,ky)][cin, kz*24+o], moving x[cin, y-tile, zpad-window]
    -> PSUM[(kz*24+o), y, zpad].  Stationary is padded to 128 columns so the
    fast-weight-load path engages.
  - fixup per tile (all off the tensor engine): PSUM->SBUF fp16 cast copy
    (ACT), 5 z-shift SBUF->SBUF DMAs into 24-partition tiles (SWDGE; DMA APs
    are exempt from the engines' 32-partition alignment rule), a 4-deep DVE
    add chain for the z-tap sum, sigmoid on the 24 gate partitions (ACT),
    and a gate->channel broadcast as two free-dim stride-0 replicating DMAs
    on the sync engine's HWDGE ring.
  - gating multiply on DVE over partitions 0:88, relu of the scalar block at
    partitions 96:128 on ACT, 3 channel-block output DMAs per x-plane that
    un-permute on the way out.

vs the previous version this removes the per-tile esum selector matmul (and
its gpre PSUM banks) from the tensor engine, which measures ~15 us faster;
the conv matmul stream itself is the hard floor (the PE sustains ~2.0-2.2
GHz under full 8-core load, so the 72x25xN=416 fp16 stream is ~375-390 us).
"""

import sys

if "/opt/trn_rl_repo" not in sys.path:
    sys.path.insert(0, "/opt/trn_rl_repo")

import numpy as np

B = 2
C = 120
S = 48          # spatial size
K = 5           # conv kernel size (odd)
PAD = 2
NXS = 4         # x-axis shards
XS = S // NXS   # 12 output x-planes per core
XIN = XS + 2 * PAD   # 16 input planes per core
SP = S + 2 * PAD     # 52, padded y/z extent
YT = 8          # y-tile rows per matmul
NYT = S // YT   # 6 y-tiles per plane
NCO = 24        # gate output channels
NSC = 32        # scalar (relu) channels
CP = 128        # stationary columns incl. pad (FWL wants 128)
CIN = 128       # contraction rows incl. pad (8 zero input channels)
N_CORES = 8
NG = 88         # gated channels in permuted order (48 l1 + 40 l2)
SC0 = 96        # permuted base of the scalar block

_CACHE = {}


def _build(reps=1):
    import contextlib

    import concourse.tile as tile
    from concourse import bacc, mybir

    f32 = mybir.dt.float32
    f16 = mybir.dt.float16

    nc = bacc.Bacc("TRN2", target_bir_lowering=False, debug=False,
                   num_devices=N_CORES)
    xs_d = nc.dram_tensor("xs", [CIN, XIN, SP, SP], f16, kind="ExternalInput").ap()
    wst_d = nc.dram_tensor("wst", [CIN, K * K, CP], f16, kind="ExternalInput").ap()
    y_d = nc.dram_tensor("y", [C, XS, S, S], f32, kind="ExternalOutput").ap()

    with tile.TileContext(nc) as tc:
        with tc.tile_pool(name="wpool", bufs=1) as wpool, \
             tc.tile_pool(name="planes", bufs=8) as plane_pool, \
             tc.tile_pool(name="convps", bufs=6, space="PSUM") as conv_pool, \
             tc.tile_pool(name="sshift", bufs=3) as sshift_pool, \
             tc.tile_pool(name="stap", bufs=3) as stap_pool, \
             tc.tile_pool(name="gs24p", bufs=3) as gs24_pool, \
             tc.tile_pool(name="gsigp", bufs=3) as gsig_pool, \
             tc.tile_pool(name="outpl", bufs=2) as out_pool:

            wst_t = wpool.tile([CIN, K * K, CP], f16)
            nc.sync.dma_start(wst_t[:], wst_d[:])

            planes = {}
            planes_f = {}

            def load_plane(s):
                t = plane_pool.tile([CIN, SP, SP], f16, tag="plane", name=f"plane{s}")
                nc.sync.dma_start(t[:], xs_d[:, s])
                planes[s] = t
                planes_f[s] = t[:].rearrange("p y z -> p (y z)")

            outplanes = {}

            def emit_fixup(p, k, cps):
                # PSUM -> SBUF cast copy (fp16)
                ss = sshift_pool.tile([CIN, YT, SP], f16)
                nc.scalar.copy(ss[:].rearrange("p y z -> p (y z)"), cps[:, :])
                # 5 parallel z-shift DMAs to 24-partition tiles, then a
                # 4-deep DVE add chain (in-place) for the z-tap sum.
                staps = []
                for c in range(K):
                    t = stap_pool.tile([NCO, YT, S], f16, tag=f"stap{c}",
                                       name=f"stap{c}")
                    nc.gpsimd.dma_start(
                        t[:], ss[NCO * c:NCO * (c + 1), :, c:c + S])
                    staps.append(t)
                stap = staps[0]
                for c in range(1, K):
                    nc.vector.tensor_add(stap[:], stap[:], staps[c][:])
                gs24 = gs24_pool.tile([NCO, YT, S], f16)
                nc.scalar.activation(gs24[:], stap[:],
                                     mybir.ActivationFunctionType.Sigmoid)
                # gate -> channel broadcast (replicate 3x for l=1, 5x for l=2)
                gsig = gsig_pool.tile([NG, YT, S], f16)
                rep1 = gs24[0:16].rearrange("m y z -> m (y z)") \
                    .unsqueeze(1).broadcast_to([16, 3, YT * S])
                nc.sync.dma_start(gsig[0:48].rearrange("p y z -> p (y z)"), rep1)
                rep2 = gs24[16:24].rearrange("m y z -> m (y z)") \
                    .unsqueeze(1).broadcast_to([8, 5, YT * S])
                nc.sync.dma_start(gsig[48:88].rearrange("p y z -> p (y z)"), rep2)

                if k == 0:
                    outplanes[p] = out_pool.tile([CIN, S, S], f32, tag="outplane",
                                                 name=f"outplane{p}")
                op_t = outplanes[p]
                xc = planes[p + PAD]  # center plane (kx = PAD)
                ys = k * YT
                nc.vector.tensor_mul(
                    op_t[0:NG, ys:ys + YT, :],
                    xc[0:NG, ys + PAD:ys + PAD + YT, PAD:PAD + S],
                    gsig[:, :, :])
                nc.scalar.activation(
                    op_t[SC0:CIN, ys:ys + YT, :],
                    xc[SC0:CIN, ys + PAD:ys + PAD + YT, PAD:PAD + S],
                    mybir.ActivationFunctionType.Relu)
                if k == NYT - 1:
                    # un-permute channel blocks on the way out
                    nc.sync.dma_start(y_d[NSC:NSC + 48, p], op_t[0:48])
                    nc.sync.dma_start(y_d[NSC + 48:C, p], op_t[48:NG])
                    nc.sync.dma_start(y_d[0:NSC, p], op_t[SC0:SC0 + NSC])
                    del outplanes[p]

            rep_ctx = tc.For_i(0, reps, 1) if reps > 1 else contextlib.nullcontext()
            with rep_ctx:
                for s in range(K):
                    load_plane(s)

                pending = []
                for p in range(XS):
                    for k in range(NYT):
                        if k == 0 and p + K < XIN:
                            load_plane(p + K)
                        cps = conv_pool.tile([CP, YT * SP], f32)
                        for a in range(K):
                            for b in range(K):
                                st = (k * YT + b) * SP
                                nc.tensor.matmul(
                                    cps[:],
                                    wst_t[:, a * K + b, :],
                                    planes_f[p + a][:, st:st + YT * SP],
                                    start=(a == 0 and b == 0),
                                    stop=(a == K - 1 and b == K - 1))
                        pending.append((p, k, cps))
                        if len(pending) > 2:
                            emit_fixup(*pending.pop(0))
                for args in pending:
                    emit_fixup(*args)

    nc.compile()
    return nc


# permuted channel order: [32:80 (l1), 80:120 (l2)] -> 0:88, scalars -> 96:128
_PERM = np.concatenate([np.arange(32, 80), np.arange(80, 120)])


def _host_inputs(x, w_gate):
    """Build the 8 per-core input maps (matmul operands pre-cast to fp16)."""
    x = np.ascontiguousarray(x, dtype=np.float32)
    w_gate = np.ascontiguousarray(w_gate, dtype=np.float32)

    # stationary: rows permuted like the slab; Wst[i', a*K+b, c*24+o]
    wst = np.transpose(w_gate, (1, 2, 3, 4, 0)).reshape(C, K * K, K * NCO)
    wstp = np.zeros((CIN, K * K, CP), dtype=np.float16)
    wstp[0:NG, :, :K * NCO] = wst[_PERM].astype(np.float16)
    wstp[SC0:SC0 + NSC, :, :K * NCO] = wst[0:NSC].astype(np.float16)

    in_maps = []
    for i in range(N_CORES):
        b = i // NXS
        x0 = (i % NXS) * XS
        slab = np.zeros((CIN, XIN, SP, SP), dtype=np.float16)
        s0 = max(0, x0 - PAD)
        s1 = min(S, x0 + XS + PAD)
        d0 = s0 - (x0 - PAD)
        xx = x[b, :, s0:s1].astype(np.float16)
        slab[0:NG, d0:d0 + (s1 - s0), PAD:PAD + S, PAD:PAD + S] = xx[_PERM]
        slab[SC0:SC0 + NSC, d0:d0 + (s1 - s0), PAD:PAD + S, PAD:PAD + S] = xx[0:NSC]
        in_maps.append({"xs": slab, "wst": wstp})
    return in_maps


def kernel(x, w_gate):
    import time

    from concourse.bass_utils import run_bass_kernel_spmd

    if "nc" not in _CACHE:
        _CACHE["nc"] = _build()
    nc = _CACHE["nc"]

    in_maps = _host_inputs(x, w_gate)
    last_err = None
    for attempt in range(3):
        try:
            res = run_bass_kernel_spmd(nc, in_maps, core_ids=list(range(N_CORES)))
            break
        except Exception as e:  # transient NRT device wedges recover on retry
            last_err = e
            time.sleep(5.0)
    else:
        raise last_err
    kernel._last_results = res

    out = np.empty((B, C, S, S, S), dtype=np.float32)
    for i in range(N_CORES):
        b = i // NXS
        x0 = (i % NXS) * XS
        out[b, :, x0:x0 + XS] = res.results[i]["y"]
    return out
